# revision 45
# baseline (speedup 1.0000x reference)
"""BAM-style attention block (avgpool8 -> 1024-token attention -> nearest-upsample + residual)
as a distributed Bass kernel on 8 TRN2 NeuronCores.

Sharding: core = b*2 + half  (b = batch 0..3, half = H-half 0..1).
x streams through HBM in fp16 (host casts; the output is dominated by the
residual x, so fp16's ~5e-4 element error is far inside the 2e-2 envelope).

Each core:
  phase 1: streams its x shard [512, 128, 256] in 2MB tiles (sync ring),
           avg-pools 8x8 on DVE, and pipelines a pairwise AllGather of each
           pooled 128-channel group (gpsimd ring) with the streaming
  phase 2: q/k/v projections + 512x1024 attention (bf16) in local-first token
           order. Energies are computed TRANSPOSED (eT[n,m] = k^T q via one
           matmul per 128-token source tile) so exp() lands directly in the
           layout the y-contraction needs - no PE transposes on the critical
           path. Softmax normalization is deferred to a final row-sum rescale.
  phase 3: re-streams x in 2MB tiles (sync ring), adds the upsampled
           attention output on DVE, writes out (scalar ring). Six tile
           buffers let ~12MB of loads prefetch during the phase-2 gap.

All element-wise traffic (pool reduce, residual add) keeps every DVE operand
2-byte, packed-innermost and in SBUF, which qualifies for the DVE 2x/4x fast
modes - without this the DVE (~1.1ns/elem/partition at 1x) paces both phases.
"""

import os
import numpy as np

B, C, H, W = 4, 512, 256, 256
DS = 8
HL = H // 2            # 128 rows per core
IL = HL // DS          # 16 pooled rows per core
WP = W // DS           # 32 pooled cols
NLOC = IL * WP         # 512 local tokens
N = 2 * NLOC           # 1024 tokens
K = C // 8             # 64
CG = C // 128          # 4 channel groups
NT = N // 128          # 8 token tiles (0..3 local, 4..7 remote)
TROWS = 32             # x rows per streaming tile (= 4 pooled rows)
P1T = HL // TROWS      # 4 tiles per channel group in phase 1/3
QTOK = 4 * WP          # 128 tokens staged per tile (AllGather quarter)

_CACHE = {}
TRACE = bool(int(os.environ.get("BAM_TRACE", "0")))
LAST_EXEC_NS = None


def _build():
    import concourse.bass as bass
    import concourse.tile as tile
    from concourse import bacc, mybir
    from concourse.masks import make_identity

    f32 = mybir.dt.float32
    f16 = mybir.dt.float16
    bf16 = mybir.dt.bfloat16
    ADD = mybir.AluOpType.add
    SUB = mybir.AluOpType.subtract
    MUL = mybir.AluOpType.mult
    AXY = mybir.AxisListType.XY
    Exp = mybir.ActivationFunctionType.Exp
    POOL_SCALE = 1.0 / (DS * DS)

    nc = bacc.Bacc("TRN2", target_bir_lowering=False, debug=False, num_devices=8)

    x_ext = nc.dram_tensor("x", [C, HL, W], f16, kind="ExternalInput")
    wq_ext = nc.dram_tensor("wq", [K, C], f32, kind="ExternalInput")
    bq_ext = nc.dram_tensor("bq", [1, K], f32, kind="ExternalInput")
    wk_ext = nc.dram_tensor("wk", [K, C], f32, kind="ExternalInput")
    bk_ext = nc.dram_tensor("bk", [1, K], f32, kind="ExternalInput")
    wv_ext = nc.dram_tensor("wv", [C, C], f32, kind="ExternalInput")
    bv_ext = nc.dram_tensor("bv", [1, C], f32, kind="ExternalInput")
    out_ext = nc.dram_tensor("out", [C, HL, W], f16, kind="ExternalOutput")

    with tile.TileContext(nc) as tc:
        with tc.tile_pool(name="persist", bufs=1) as persist, \
             tc.tile_pool(name="scratch", bufs=2) as scratch, \
             tc.tile_pool(name="p1", bufs=2) as p1, \
             tc.tile_pool(name="p3", bufs=5) as p3, \
             tc.tile_pool(name="psA", bufs=4, space="PSUM") as psA, \
             tc.tile_pool(name="psY", bufs=1, space="PSUM") as psY, \
             tc.tile_pool(name="dram", bufs=1, space="DRAM") as dram:

            # ---- constants & weights (scalar-engine DMA ring; PE transposes) ----
            ident = persist.tile([128, 128], bf16, tag="ident")
            make_identity(nc, ident[:])
            ones = persist.tile([1, N], bf16, tag="ones")
            nc.vector.memset(ones[:], 1.0)
            ones_col = persist.tile([128, 1], bf16, tag="ones_col")
            nc.vector.memset(ones_col[:], 1.0)
            ones_f32 = persist.tile([1, 128], f32, tag="ones_f32")
            nc.vector.memset(ones_f32[:], 1.0)

            def load_bias(ext, n):
                st = scratch.tile([1, n], f32, tag="bstage")
                nc.scalar.dma_start(out=st[:], in_=ext.ap())
                bb = persist.tile([1, n], bf16, tag=f"b_{ext.name}", name=f"b_{ext.name}")
                nc.scalar.copy(out=bb[:], in_=st[:])
                return bb

            bq_b = load_bias(bq_ext, K)
            bk_b = load_bias(bk_ext, K)
            bv_b = load_bias(bv_ext, C)
            # doubled biases for the pair-sum algebra: W*hsb + 2b - (W*xf + b)
            # = W*xf_partner + b
            bk2_b = persist.tile([1, K], bf16, tag="bk2")
            nc.vector.tensor_scalar_mul(bk2_b[:], bk_b[:], 2.0)
            bv2_b = persist.tile([1, C], bf16, tag="bv2")
            nc.vector.tensor_scalar_mul(bv2_b[:], bv_b[:], 2.0)

            def load_qk_weight(ext):
                st = scratch.tile([K, C], f32, tag="wstage")
                nc.scalar.dma_start(out=st[:], in_=ext.ap())
                wb = persist.tile([K, C], bf16, tag=f"wb_{ext.name}", name=f"wb_{ext.name}")
                nc.scalar.copy(out=wb[:], in_=st[:])
                wT = []
                for cg in range(CG):
                    ps = psA.tile([128, K], bf16, tag="s")
                    nc.tensor.transpose(ps[:], wb[:, cg * 128:(cg + 1) * 128],
                                        ident[0:K, 0:K])
                    t = persist.tile([128, K], bf16, tag=f"wT_{ext.name}{cg}",
                                     name=f"wT_{ext.name}{cg}")
                    nc.scalar.copy(out=t[:], in_=ps[:])
                    wT.append(t)
                return wT

            wqT = load_qk_weight(wq_ext)
            wkT = load_qk_weight(wk_ext)

            # wvT[cg][c_loc, d] = Wv[d, cg*128 + c_loc]
            wvT = [persist.tile([128, C], bf16, tag=f"wvT{cg}", name=f"wvT{cg}")
                   for cg in range(CG)]
            for dt in range(CG):
                st = scratch.tile([128, C], f32, tag="wstage")
                nc.scalar.dma_start(out=st[:], in_=wv_ext.ap()[dt * 128:(dt + 1) * 128, :])
                wvb = scratch.tile([128, C], bf16, tag="wvstage")
                nc.scalar.copy(out=wvb[:], in_=st[:])
                for cg in range(CG):
                    ps = psA.tile([128, 128], bf16, tag="s")
                    nc.tensor.transpose(ps[:], wvb[:, cg * 128:(cg + 1) * 128], ident[:])
                    nc.scalar.copy(out=wvT[cg][:, dt * 128:(dt + 1) * 128], in_=ps[:])

            # ---- phase 1: stream x + avg-pool; per-cg exchange on the gpsimd ring ----
            # Tokens are kept LOCAL-FIRST through phase 2: token tiles 0..3 are this
            # core's, 4..7 the partner's. Softmax and the final contraction are
            # permutation-invariant over n, so the global order is never
            # materialized.
            # Pooling runs in fp16 end-to-end: raw 8x8 block sums are ~N(0,64)
            # (|sum| < ~40), comfortably inside fp16, and keeping every DVE
            # operand 2-byte/packed/SBUF enables the 2x/4x DVE fast modes -
            # the pool reduce is the phase-1 pacing item otherwise.
            xf = [persist.tile([128, NLOC], f16, tag=f"xf{cg}", name=f"xf{cg}")
                  for cg in range(CG)]
            xfb_loc = [persist.tile([128, NLOC], bf16, tag=f"xfl{cg}", name=f"xfl{cg}")
                       for cg in range(CG)]
            # hsb[cg] = (h0 + h1) * POOL_SCALE in bf16: the pair-summed pooled
            # activations straight off the AllReduce
            hsb = [persist.tile([128, NLOC], bf16, tag=f"hsb{cg}", name=f"hsb{cg}")
                   for cg in range(CG)]
            # staging is packed per cg-PAIR: two collectives total. Each
            # costs ~15-25us of latency + inter-op dead time on the cc stream
            # regardless of size, so fewer, larger exchanges win; the first
            # fires at P1 halftime (hidden), only the second is exposed.
            # AllReduce(add) delivers h0+h1 directly, so the partner half is
            # recovered with a single subtract and half the readback of an
            # AllGather.
            xf_loc_d = dram.tile([2, 128, 2 * NLOC], f16, tag="xf_loc")
            hsum_d = dram.tile([2, 128, 2 * NLOC], f16, tag="hsum_d")

            q_ps = psA.tile([K, NLOC], f32, tag="s")
            kl_ps = psA.tile([K, NLOC], f32, tag="s")
            kr_ps = psA.tile([K, NLOC], f32, tag="s")

            def recover_pair(pr):
                # No explicit partner-half recovery: k and v are LINEAR in the
                # pooled input, so the remote contributions are derived from
                # the AllReduce sum algebraically (k_rem = Wk*hsb + 2bk -
                # k_loc, likewise per v-tile). hsb is the scaled pair-sum; the
                # only per-half arithmetic left is one tiny subtract at the
                # point of use, and the gpsimd stream carries nothing but
                # staging DMAs and the two collective triggers (a gpsimd
                # compute op here costs a Q7 library swap and lets the Tile
                # scheduler head-of-line block the second collective).
                for cg in (2 * pr, 2 * pr + 1):
                    sl = (cg % 2) * NLOC
                    hs = scratch.tile([128, NLOC], f16, tag="hsum",
                                      name=f"hs{cg}")
                    nc.scalar.dma_start(out=hs[:],
                                        in_=hsum_d[pr][:, sl:sl + NLOC])
                    nc.scalar.activation(out=hsb[cg][:], in_=hs[:],
                                         func=mybir.ActivationFunctionType.Copy,
                                         scale=POOL_SCALE)
                    nc.tensor.matmul(kr_ps[:], wkT[cg][:], hsb[cg][:],
                                     start=(cg == 0), stop=False)

            lp = nc.allow_low_precision("8x8 block sums are ~N(0,64); fp16 "
                                        "keeps DVE in its 2-byte fast mode")
            lp.__enter__()
            HT = TROWS // 2  # 16, 8, 4: pairwise row-sum tree widths
            for cg in range(CG):
                for it in range(P1T):
                    x1 = p1.tile([128, TROWS, W], f16, tag="x1", bufs=3)
                    nc.sync.dma_start(
                        out=x1[:],
                        in_=x_ext.ap()[cg * 128:(cg + 1) * 128,
                                       it * TROWS:(it + 1) * TROWS, :])
                    # 8x8 block sum as a pairwise row tree (TENSOR_TENSOR runs
                    # in the DVE 2-byte fast mode; TENSOR_REDUCE does not) plus
                    # one small strided reduce over the 8-column groups.
                    s1 = p1.tile([128, HT, W], f16, tag="s1", bufs=1,
                                 name=f"s1_{cg}_{it}")
                    v0 = x1[:].rearrange("p (a b) w -> p a b w", b=2)
                    nc.vector.tensor_tensor(out=s1[:], in0=v0[:, :, 0, :],
                                            in1=v0[:, :, 1, :], op=ADD)
                    s2 = p1.tile([128, HT // 2, W], f16, tag="s2", bufs=1,
                                 name=f"s2_{cg}_{it}")
                    v1 = s1[:].rearrange("p (a b) w -> p a b w", b=2)
                    nc.vector.tensor_tensor(out=s2[:], in0=v1[:, :, 0, :],
                                            in1=v1[:, :, 1, :], op=ADD)
                    s3 = p1.tile([128, HT // 4, W], f16, tag="s3", bufs=1,
                                 name=f"s3_{cg}_{it}")
                    v2 = s2[:].rearrange("p (a b) w -> p a b w", b=2)
                    nc.vector.tensor_tensor(out=s3[:], in0=v2[:, :, 0, :],
                                            in1=v2[:, :, 1, :], op=ADD)
                    nc.vector.tensor_reduce(
                        out=xf[cg][:, it * QTOK:(it + 1) * QTOK]
                            .rearrange("p (r j) -> p r j", j=WP),
                        in_=s3[:].rearrange("p r (j z) -> p r j z", z=DS),
                        axis=mybir.AxisListType.X, op=ADD)
                    # stage each completed quarter into the bounce buffer so
                    # the collective fires the moment the last slice lands
                    nc.gpsimd.dma_start(
                        out=xf_loc_d[cg // 2][:, (cg % 2) * NLOC
                                              + it * QTOK:(cg % 2) * NLOC
                                              + (it + 1) * QTOK],
                        in_=xf[cg][:, it * QTOK:(it + 1) * QTOK])

                # local bf16 copy (applies the 1/64 pooling scale); on DVE so it
                # slots right behind this group's own pooling ADDs
                nc.vector.tensor_scalar_mul(xfb_loc[cg][:], xf[cg][:], POOL_SCALE)
                # local q/k partials (overlap the exchange)
                nc.tensor.matmul(q_ps[:], wqT[cg][:], xfb_loc[cg][:],
                                 start=(cg == 0), stop=False)
                nc.tensor.matmul(kl_ps[:], wkT[cg][:], xfb_loc[cg][:],
                                 start=(cg == 0), stop=False)
                if cg % 2 == 1:
                    nc.gpsimd.collective_compute(
                        "AllReduce",
                        ADD,
                        ins=[xf_loc_d[cg // 2].opt()],
                        outs=[hsum_d[cg // 2].opt()],
                        replica_groups=[[0, 1], [2, 3], [4, 5], [6, 7]],
                    )

            lp.__exit__(None, None, None)
            recover_pair(0)

            # ================= LOCAL attention half =================
            # Everything below up to the "REMOTE" marker depends only on local
            # pooled data, so it executes while the second AllReduce is in
            # flight. high_priority pins it early in every engine stream so
            # the scheduler cannot wedge collective-dependent pair-1 work in
            # front of it (dependencies still keep it after the pooling).
            hp = tc.high_priority()
            hp.__enter__()
            nc.tensor.matmul(q_ps[:], bq_b[:], ones[:, :NLOC], start=False, stop=True)
            q_sb = persist.tile([K, NLOC], bf16, tag="q_sb")
            nc.vector.tensor_copy(out=q_sb[:], in_=q_ps[:])
            nc.tensor.matmul(kl_ps[:], bk_b[:], ones[:, :NLOC], start=False, stop=True)
            k_loc = persist.tile([K, NLOC], bf16, tag="k_loc")
            nc.vector.tensor_copy(out=k_loc[:], in_=kl_ps[:])

            vT = [persist.tile([128, C], bf16, tag=f"vT{nt}", name=f"vT{nt}")
                  for nt in range(NT)]
            attnT = [persist.tile([128, NLOC], bf16, tag=f"attnT{nt}",
                                  name=f"attnT{nt}")
                     for nt in range(NT)]
            y_ps = [psY.tile([128, NLOC], f32, tag=f"y{dt}", name=f"yps{dt}")
                    for dt in range(CG)]
            rs_ps = psA.tile([1, NLOC], f32, tag="s", name="rs_ps")

            def vt_tile(nt):
                # vT[nt][n, d] = v[d, n] for token tile nt (128 tokens).
                # Remote tiles come from the pair-sum: v_sum(+2bv) minus the
                # already-computed local tile (fused into the PSUM drain).
                src = xfb_loc if nt < 4 else hsb
                bias = bv_b if nt < 4 else bv2_b
                j = nt % 4
                v_ps = psA.tile([128, C], f32, tag="s", name=f"v_ps{nt}")
                for cg in range(CG):
                    nc.tensor.matmul(v_ps[:], src[cg][:, j * 128:(j + 1) * 128],
                                     wvT[cg][:], start=(cg == 0), stop=False)
                nc.tensor.matmul(v_ps[:], ones[:, :128], bias[:], start=False, stop=True)
                if nt < 4:
                    nc.vector.tensor_copy(out=vT[nt][:], in_=v_ps[:])
                else:
                    nc.vector.tensor_tensor(out=vT[nt][:], in0=v_ps[:],
                                            in1=vT[nt - 4][:], op=SUB)

            # attnT holds UNNORMALIZED exp(eT/sqrt(K)); normalization is applied
            # to y at the very end via a row-sum rescale. Energies are tiny for
            # this model (|e/sqrt(K)| < ~0.05), so exp without max-subtraction is
            # safe. eT[n, m] = sum_K k[K, n] q[K, m] comes out of the PE already
            # transposed, exactly the layout the y-contraction consumes.
            def attn_tile(nt):
                ksb = k_loc if nt < 4 else k_rem
                j = nt % 4
                e_ps = psA.tile([128, NLOC], f32, tag="s", name=f"e_ps{nt}")
                nc.tensor.matmul(e_ps[:], ksb[:, j * 128:(j + 1) * 128], q_sb[:],
                                 start=True, stop=True)
                nc.scalar.activation(out=attnT[nt][:], in_=e_ps[:], func=Exp,
                                     scale=K ** -0.5)

            def y_accum(nt):
                # y_raw[d, m] += sum_n v[d, n] a[m, n]; rowsum[m] += sum_n a[m, n]
                for dt in range(CG):
                    nc.tensor.matmul(y_ps[dt][:], vT[nt][:, dt * 128:(dt + 1) * 128],
                                     attnT[nt][:], start=(nt == 0),
                                     stop=(nt == NT - 1))
                nc.tensor.matmul(rs_ps[:], ones_col[:], attnT[nt][:],
                                 start=(nt == 0), stop=(nt == NT - 1))

            for nt in range(4):
                vt_tile(nt)
                attn_tile(nt)
                y_accum(nt)
            hp.__exit__(None, None, None)

            # ================= REMOTE attention half =================
            recover_pair(1)
            nc.tensor.matmul(kr_ps[:], bk2_b[:], ones[:, :NLOC], start=False,
                             stop=True)
            k_rem = persist.tile([K, NLOC], bf16, tag="k_rem")
            nc.vector.tensor_tensor(out=k_rem[:], in0=kr_ps[:], in1=k_loc[:],
                                    op=SUB)

            for nt in range(4, NT):
                vt_tile(nt)
                attn_tile(nt)
                y_accum(nt)

            # softmax denominators -> broadcast rescale of y. The [1,512] row
            # is serial work on one DVE lane; the ~18-bit fast approximation
            # (5x faster than full reciprocal) is far more accurate than this
            # kernel needs (rowsums are ~1024, inputs well-conditioned).
            rinv_row = persist.tile([1, NLOC], f32, tag="rinv_row")
            nc.vector.reciprocal_approx_fast(out=rinv_row[:], in_=rs_ps[:])
            rb_ps = psA.tile([128, NLOC], f32, tag="s")
            nc.tensor.matmul(rb_ps[:], ones_f32[:], rinv_row[:], start=True, stop=True)
            rb_sb = persist.tile([128, NLOC], f32, tag="rb_sb")
            nc.vector.tensor_copy(out=rb_sb[:], in_=rb_ps[:])

            y = [persist.tile([128, NLOC], f16, tag=f"y{dt}", name=f"y{dt}")
                 for dt in range(CG)]
            for dt in range(CG):
                nc.vector.tensor_tensor(out=y[dt][:], in0=y_ps[dt][:], in1=rb_sb[:],
                                        op=MUL)

            # ---- phase 3: out = x + upsample8(y) ----
            # loads on sync ring, adds on DVE, stores on scalar ring.
            # yup[cg] is y upsampled 8x along W (built once per cg on the
            # otherwise-idle ACT engine); the DVE adds then broadcast it only
            # across rows, keeping the innermost axis packed so the adds run
            # in the DVE 2-byte fast mode.
            for cg in range(CG):
                yup = p1.tile([128, IL, W], f16, tag="x1", bufs=3,
                              name=f"yup{cg}")
                nc.scalar.copy(
                    out=yup[:].rearrange("p i (j z) -> p i j z", z=DS),
                    in_=y[cg][:].rearrange("p (i j) -> p i j", i=IL)
                        [:, :, :, None].broadcast_to([128, IL, WP, DS]))
                for it in range(P1T):
                    x3 = p3.tile([128, TROWS, W], f16, tag="x3")
                    nc.sync.dma_start(
                        out=x3[:],
                        in_=x_ext.ap()[cg * 128:(cg + 1) * 128,
                                       it * TROWS:(it + 1) * TROWS, :])
                    for i in range(TROWS // DS):
                        xv = x3[:, i * DS:(i + 1) * DS, :]
                        yv = yup[:, it * 4 + i:it * 4 + i + 1, :] \
                            .broadcast_to([128, DS, W])
                        nc.vector.tensor_tensor(out=xv, in0=xv, in1=yv, op=ADD)
                    nc.scalar.dma_start(
                        out=out_ext.ap()[cg * 128:(cg + 1) * 128,
                                         it * TROWS:(it + 1) * TROWS, :],
                        in_=x3[:])

    nc.finalize()
    return nc


def _get_nc():
    if "nc" not in _CACHE:
        _CACHE["nc"] = _build()
    return _CACHE["nc"]


def kernel(x, Wq, bq, Wk, bk, Wv, bv):
    global LAST_EXEC_NS
    from concourse.bass_utils import run_bass_kernel_spmd

    # x round-trips HBM three times (pool read, residual read, output
    # write); fp16 halves that traffic and its ~5e-4 element error is far
    # inside the output envelope (the attention correction is ~1e-2 of the
    # residual magnitude, and the tolerance is 2e-2).
    x = np.asarray(x, dtype=np.float32).astype(np.float16)
    Wq = np.asarray(Wq, dtype=np.float32)
    bq = np.asarray(bq, dtype=np.float32).reshape(1, K)
    Wk = np.asarray(Wk, dtype=np.float32)
    bk = np.asarray(bk, dtype=np.float32).reshape(1, K)
    Wv = np.asarray(Wv, dtype=np.float32)
    bv = np.asarray(bv, dtype=np.float32).reshape(1, C)

    nc = _get_nc()
    in_maps = []
    for core in range(8):
        b, half = core // 2, core % 2
        in_maps.append({
            "x": np.ascontiguousarray(x[b, :, half * HL:(half + 1) * HL, :]),
            "wq": Wq, "bq": bq, "wk": Wk, "bk": bk, "wv": Wv, "bv": bv,
        })

    res = run_bass_kernel_spmd(nc, in_maps, core_ids=list(range(8)), trace=TRACE,
                               tmpdir=os.environ.get("BAM_TMPDIR") or None)
    LAST_EXEC_NS = res.exec_time_ns

    out = np.empty((B, C, H, W), dtype=np.float32)
    for core in range(8):
        b, half = core // 2, core % 2
        out[b, :, half * HL:(half + 1) * HL, :] = res.results[core]["out"]
    return out


# revision 46
# speedup vs baseline: 1.0328x; 1.0328x over previous
"""BAM-style attention block (avgpool8 -> 1024-token attention -> nearest-upsample + residual)
as a distributed Bass kernel on 8 TRN2 NeuronCores.

Sharding: core = b*2 + half  (b = batch 0..3, half = H-half 0..1).
x streams through HBM in fp16 (host casts; the output is dominated by the
residual x, so fp16's ~5e-4 element error is far inside the 2e-2 envelope).

Per core:
  phase 1: streams its x shard [512, 128, 256] in 2MB tiles (sync ring) and
           8x8-pools on DVE as a pairwise row-sum tree (TENSOR_TENSOR stays in
           the DVE 2-byte fast mode; TENSOR_REDUCE does not) + one small
           column-group reduce. Raw block SUMS (~N(0,64)) are kept in fp16;
           the 1/64 pool scale is folded into the q/k/v weight tiles, so the
           pooled data is consumed by the PE with no rescale pass at all.
  exchange: two pairwise AllGathers of the raw pooled sums (one per
           channel-group pair; each collective costs ~20us regardless of
           size, so the first is fired at P1 halftime and hides, only the
           second is exposed). The partner half is never materialized:
           k/v are linear in the pooled input, so remote tiles come from
           matmul-accumulating BOTH gathered halves (+2b bias) and
           subtracting the already-computed local tile during the PSUM
           drain - the gpsimd stream carries nothing but staging DMAs and
           collective triggers (a gpsimd compute op costs a Q7 library swap
           and lets the Tile scheduler head-of-line block the collectives).
  phase 2: 512x1024 attention, all-fp16 operands. Energies are computed
           TRANSPOSED (eT[n,m] = k^T q, one matmul per 128-token source tile)
           so exp() lands directly in the layout the y-contraction needs -
           no PE transposes. Softmax normalization is deferred to a row-sum
           rescale of y. The local token half is emitted (and scheduled)
           ahead of everything exchange-dependent.
  phase 3: re-streams x in 2MB tiles, adds the upsampled attention output on
           DVE (y is pre-upsampled along W on the idle ACT engine so the DVE
           add keeps a packed innermost axis), writes out (scalar ring).
           Phase-3 tiles share the phase-1 tile slots, which both bounds SBUF
           and naturally delays the prefetch until pooling drains - the
           loads then fill the exchange-latency gap instead of competing
           with phase-1 streaming.
"""

import os
import numpy as np

B, C, H, W = 4, 512, 256, 256
DS = 8
HL = H // 2            # 128 rows per core
IL = HL // DS          # 16 pooled rows per core
WP = W // DS           # 32 pooled cols
NLOC = IL * WP         # 512 local tokens
N = 2 * NLOC           # 1024 tokens
K = C // 8             # 64
CG = C // 128          # 4 channel groups
NT = N // 128          # 8 token tiles (0..3 local, 4..7 remote)
TROWS = 32             # x rows per streaming tile (= 4 pooled rows)
P1T = HL // TROWS      # 4 tiles per channel group in phase 1/3
QTOK = 4 * WP          # 128 tokens staged per tile (exchange quarter)

_CACHE = {}
TRACE = bool(int(os.environ.get("BAM_TRACE", "0")))
LAST_EXEC_NS = None


def _build():
    import concourse.bass as bass
    import concourse.tile as tile
    from concourse import bacc, mybir
    from concourse.masks import make_identity

    f32 = mybir.dt.float32
    f16 = mybir.dt.float16
    ADD = mybir.AluOpType.add
    SUB = mybir.AluOpType.subtract
    MUL = mybir.AluOpType.mult
    Copy = mybir.ActivationFunctionType.Copy
    Exp = mybir.ActivationFunctionType.Exp
    POOL_SCALE = 1.0 / (DS * DS)

    nc = bacc.Bacc("TRN2", target_bir_lowering=False, debug=False, num_devices=8)

    x_ext = nc.dram_tensor("x", [C, HL, W], f16, kind="ExternalInput")
    wq_ext = nc.dram_tensor("wq", [K, C], f32, kind="ExternalInput")
    bq_ext = nc.dram_tensor("bq", [1, K], f32, kind="ExternalInput")
    wk_ext = nc.dram_tensor("wk", [K, C], f32, kind="ExternalInput")
    bk_ext = nc.dram_tensor("bk", [1, K], f32, kind="ExternalInput")
    wv_ext = nc.dram_tensor("wv", [C, C], f32, kind="ExternalInput")
    bv_ext = nc.dram_tensor("bv", [1, C], f32, kind="ExternalInput")
    out_ext = nc.dram_tensor("out", [C, HL, W], f16, kind="ExternalOutput")

    with tile.TileContext(nc) as tc:
        with tc.tile_pool(name="persist", bufs=1) as persist, \
             tc.tile_pool(name="scratch", bufs=2) as scratch, \
             tc.tile_pool(name="p1", bufs=7) as p1, \
             tc.tile_pool(name="psA", bufs=4, space="PSUM") as psA, \
             tc.tile_pool(name="psY", bufs=1, space="PSUM") as psY, \
             tc.tile_pool(name="dram", bufs=1, space="DRAM") as dram:

            # ---- constants & weights (scalar-engine DMA ring; PE transposes).
            # All weight tiles carry the 1/64 pooling scale so every consumer
            # of pooled data reads the raw fp16 block sums directly.
            ident = persist.tile([128, 128], f16, tag="ident")
            make_identity(nc, ident[:])
            ones = persist.tile([1, N], f16, tag="ones")
            nc.vector.memset(ones[:], 1.0)
            ones_col = persist.tile([128, 1], f16, tag="ones_col")
            nc.vector.memset(ones_col[:], 1.0)
            ones_f32 = persist.tile([1, 128], f32, tag="ones_f32")
            nc.vector.memset(ones_f32[:], 1.0)

            def load_bias(ext, n):
                st = scratch.tile([1, n], f32, tag="bstage")
                nc.scalar.dma_start(out=st[:], in_=ext.ap())
                bb = persist.tile([1, n], f16, tag=f"b_{ext.name}", name=f"b_{ext.name}")
                nc.scalar.copy(out=bb[:], in_=st[:])
                return bb

            bq_b = load_bias(bq_ext, K)
            bk_b = load_bias(bk_ext, K)
            bv_b = load_bias(bv_ext, C)
            # doubled biases for the pair-sum algebra:
            # W*(h0+h1)*s + 2b - (W*xf*s + b) = W*xf_partner*s + b
            bk2_b = persist.tile([1, K], f16, tag="bk2")
            nc.vector.tensor_scalar_mul(bk2_b[:], bk_b[:], 2.0)
            bv2_b = persist.tile([1, C], f16, tag="bv2")
            nc.vector.tensor_scalar_mul(bv2_b[:], bv_b[:], 2.0)

            def load_qk_weight(ext):
                st = scratch.tile([K, C], f32, tag="wstage")
                nc.scalar.dma_start(out=st[:], in_=ext.ap())
                wb = persist.tile([K, C], f16, tag=f"wb_{ext.name}", name=f"wb_{ext.name}")
                nc.scalar.copy(out=wb[:], in_=st[:])
                wT = []
                for cg in range(CG):
                    ps = psA.tile([128, K], f16, tag="s")
                    nc.tensor.transpose(ps[:], wb[:, cg * 128:(cg + 1) * 128],
                                        ident[0:K, 0:K])
                    t = persist.tile([128, K], f16, tag=f"wT_{ext.name}{cg}",
                                     name=f"wT_{ext.name}{cg}")
                    nc.scalar.activation(out=t[:], in_=ps[:], func=Copy,
                                         scale=POOL_SCALE)
                    wT.append(t)
                return wT

            wqT = load_qk_weight(wq_ext)
            wkT = load_qk_weight(wk_ext)

            # wvT[cg][c_loc, d] = Wv[d, cg*128 + c_loc] / 64
            wvT = [persist.tile([128, C], f16, tag=f"wvT{cg}", name=f"wvT{cg}")
                   for cg in range(CG)]
            for dt in range(CG):
                st = scratch.tile([128, C], f32, tag="wstage")
                nc.scalar.dma_start(out=st[:], in_=wv_ext.ap()[dt * 128:(dt + 1) * 128, :])
                wvb = scratch.tile([128, C], f16, tag="wvstage")
                nc.scalar.copy(out=wvb[:], in_=st[:])
                for cg in range(CG):
                    ps = psA.tile([128, 128], f16, tag="s")
                    nc.tensor.transpose(ps[:], wvb[:, cg * 128:(cg + 1) * 128], ident[:])
                    nc.scalar.activation(out=wvT[cg][:, dt * 128:(dt + 1) * 128],
                                         in_=ps[:], func=Copy, scale=POOL_SCALE)

            # ---- phase 1: stream x + pool; pairwise exchange on the gpsimd ring ----
            # Tokens stay LOCAL-FIRST through phase 2: token tiles 0..3 are this
            # core's, 4..7 the partner's. Softmax and the final contraction are
            # permutation-invariant over n, so the global order is never
            # materialized.
            xf = [persist.tile([128, NLOC], f16, tag=f"xf{cg}", name=f"xf{cg}")
                  for cg in range(CG)]
            # hsg[cg][hf]: the two gathered halves of the pair (raw sums); one
            # of them IS this core's xf - never disambiguated (rank-agnostic).
            hsg = [[persist.tile([128, NLOC], f16, tag=f"hsg{cg}_{hf}",
                                 name=f"hsg{cg}_{hf}") for hf in range(2)]
                   for cg in range(CG)]
            xf_loc_d = dram.tile([2, 128, 2 * NLOC], f16, tag="xf_loc")
            xf_all_d = dram.tile([2, 2, 128, 2 * NLOC], f16, tag="xf_all")

            q_ps = psA.tile([K, NLOC], f32, tag="s")
            kl_ps = psA.tile([K, NLOC], f32, tag="s")
            kr_ps = psA.tile([K, NLOC], f32, tag="s")

            lp = nc.allow_low_precision("8x8 block sums are ~N(0,64); fp16 "
                                        "keeps DVE in its 2-byte fast mode")
            lp.__enter__()
            HT = TROWS // 2  # 16, 8, 4: pairwise row-sum tree widths
            for cg in range(CG):
                for it in range(P1T):
                    x1 = p1.tile([128, TROWS, W], f16, tag="x1")
                    nc.sync.dma_start(
                        out=x1[:],
                        in_=x_ext.ap()[cg * 128:(cg + 1) * 128,
                                       it * TROWS:(it + 1) * TROWS, :])
                    s1 = p1.tile([128, HT, W], f16, tag="s1", bufs=1,
                                 name=f"s1_{cg}_{it}")
                    v0 = x1[:].rearrange("p (a b) w -> p a b w", b=2)
                    nc.vector.tensor_tensor(out=s1[:], in0=v0[:, :, 0, :],
                                            in1=v0[:, :, 1, :], op=ADD)
                    s2 = p1.tile([128, HT // 2, W], f16, tag="s2", bufs=1,
                                 name=f"s2_{cg}_{it}")
                    v1 = s1[:].rearrange("p (a b) w -> p a b w", b=2)
                    nc.vector.tensor_tensor(out=s2[:], in0=v1[:, :, 0, :],
                                            in1=v1[:, :, 1, :], op=ADD)
                    s3 = p1.tile([128, HT // 4, W], f16, tag="s3", bufs=1,
                                 name=f"s3_{cg}_{it}")
                    v2 = s2[:].rearrange("p (a b) w -> p a b w", b=2)
                    nc.vector.tensor_tensor(out=s3[:], in0=v2[:, :, 0, :],
                                            in1=v2[:, :, 1, :], op=ADD)
                    nc.vector.tensor_reduce(
                        out=xf[cg][:, it * QTOK:(it + 1) * QTOK]
                            .rearrange("p (r j) -> p r j", j=WP),
                        in_=s3[:].rearrange("p r (j z) -> p r j z", z=DS),
                        axis=mybir.AxisListType.X, op=ADD)
                    # stage each completed quarter so the collective can fire
                    # the moment the last slice lands
                    nc.gpsimd.dma_start(
                        out=xf_loc_d[cg // 2][:, (cg % 2) * NLOC
                                              + it * QTOK:(cg % 2) * NLOC
                                              + (it + 1) * QTOK],
                        in_=xf[cg][:, it * QTOK:(it + 1) * QTOK])

                # local q/k partials (rhs = raw sums; scale lives in wqT/wkT)
                nc.tensor.matmul(q_ps[:], wqT[cg][:], xf[cg][:],
                                 start=(cg == 0), stop=False)
                nc.tensor.matmul(kl_ps[:], wkT[cg][:], xf[cg][:],
                                 start=(cg == 0), stop=False)
                if cg % 2 == 1:
                    nc.gpsimd.collective_compute(
                        "AllGather",
                        mybir.AluOpType.bypass,
                        ins=[xf_loc_d[cg // 2].opt()],
                        outs=[xf_all_d[cg // 2].opt()],
                        replica_groups=[[0, 1], [2, 3], [4, 5], [6, 7]],
                    )
            lp.__exit__(None, None, None)

            # ================= LOCAL attention half =================
            # Emitted (hence per-engine scheduled) BEFORE anything that waits
            # on a collective: it runs while the second AllGather is in
            # flight.
            nc.tensor.matmul(q_ps[:], bq_b[:], ones[:, :NLOC], start=False, stop=True)
            q_sb = persist.tile([K, NLOC], f16, tag="q_sb")
            nc.vector.tensor_copy(out=q_sb[:], in_=q_ps[:])
            nc.tensor.matmul(kl_ps[:], bk_b[:], ones[:, :NLOC], start=False, stop=True)
            k_loc = persist.tile([K, NLOC], f16, tag="k_loc")
            nc.vector.tensor_copy(out=k_loc[:], in_=kl_ps[:])

            vT = [persist.tile([128, C], f16, tag=f"vT{nt}", name=f"vT{nt}")
                  for nt in range(NT)]
            attnT = [persist.tile([128, NLOC], f16, tag=f"attnT{nt}",
                                  name=f"attnT{nt}")
                     for nt in range(NT)]
            y_ps = [psY.tile([128, NLOC], f32, tag=f"y{dt}", name=f"yps{dt}")
                    for dt in range(CG)]
            rs_ps = psA.tile([1, NLOC], f32, tag="s", name="rs_ps")

            def vt_tile(nt):
                # vT[nt][n, d] = v[d, n] for token tile nt (128 tokens).
                # Remote tiles accumulate BOTH gathered halves (+2bv) and
                # subtract the local tile while draining the PSUM.
                j = nt % 4
                v_ps = psA.tile([128, C], f32, tag="s", name=f"v_ps{nt}")
                if nt < 4:
                    for cg in range(CG):
                        nc.tensor.matmul(v_ps[:], xf[cg][:, j * 128:(j + 1) * 128],
                                         wvT[cg][:], start=(cg == 0), stop=False)
                    nc.tensor.matmul(v_ps[:], ones[:, :128], bv_b[:],
                                     start=False, stop=True)
                    nc.vector.tensor_copy(out=vT[nt][:], in_=v_ps[:])
                else:
                    for cg in range(CG):
                        for hf in range(2):
                            nc.tensor.matmul(
                                v_ps[:], hsg[cg][hf][:, j * 128:(j + 1) * 128],
                                wvT[cg][:], start=(cg == 0 and hf == 0),
                                stop=False)
                    nc.tensor.matmul(v_ps[:], ones[:, :128], bv2_b[:],
                                     start=False, stop=True)
                    nc.vector.tensor_tensor(out=vT[nt][:], in0=v_ps[:],
                                            in1=vT[nt - 4][:], op=SUB)

            # attnT holds UNNORMALIZED exp(eT/sqrt(K)); normalization is a
            # final row-sum rescale of y. Energies are tiny (|e/sqrt(K)| <
            # ~0.05) so exp without max-subtraction is safe. eT[n, m] =
            # sum_K k[K, n] q[K, m] leaves the PE already transposed.
            def attn_tile(nt):
                ksb = k_loc if nt < 4 else k_rem
                j = nt % 4
                e_ps = psA.tile([128, NLOC], f32, tag="s", name=f"e_ps{nt}")
                nc.tensor.matmul(e_ps[:], ksb[:, j * 128:(j + 1) * 128], q_sb[:],
                                 start=True, stop=True)
                nc.scalar.activation(out=attnT[nt][:], in_=e_ps[:], func=Exp,
                                     scale=K ** -0.5)

            def y_accum(nt):
                # y_raw[d, m] += sum_n v[d, n] a[m, n]; rowsum[m] += sum_n a[m, n]
                for dt in range(CG):
                    nc.tensor.matmul(y_ps[dt][:], vT[nt][:, dt * 128:(dt + 1) * 128],
                                     attnT[nt][:], start=(nt == 0),
                                     stop=(nt == NT - 1))
                nc.tensor.matmul(rs_ps[:], ones_col[:], attnT[nt][:],
                                 start=(nt == 0), stop=(nt == NT - 1))

            for nt in range(4):
                vt_tile(nt)
                attn_tile(nt)
                y_accum(nt)

            # ================= REMOTE attention half =================
            def load_pair(pr):
                for cg in (2 * pr, 2 * pr + 1):
                    sl = (cg % 2) * NLOC
                    for hf in range(2):
                        nc.scalar.dma_start(
                            out=hsg[cg][hf][:],
                            in_=xf_all_d[pr, hf][:, sl:sl + NLOC])
                    for hf in range(2):
                        nc.tensor.matmul(kr_ps[:], wkT[cg][:], hsg[cg][hf][:],
                                         start=(cg == 0 and hf == 0), stop=False)

            load_pair(0)
            load_pair(1)
            nc.tensor.matmul(kr_ps[:], bk2_b[:], ones[:, :NLOC], start=False,
                             stop=True)
            k_rem = persist.tile([K, NLOC], f16, tag="k_rem")
            nc.vector.tensor_tensor(out=k_rem[:], in0=kr_ps[:], in1=k_loc[:],
                                    op=SUB)

            for nt in range(4, NT):
                vt_tile(nt)
                attn_tile(nt)
                y_accum(nt)

            # softmax denominators -> broadcast rescale of y. The [1,512] row
            # is serial one-lane work; the ~18-bit fast approximation (5x
            # faster than full reciprocal) is far more accurate than needed.
            rinv_row = persist.tile([1, NLOC], f32, tag="rinv_row")
            nc.vector.reciprocal_approx_fast(out=rinv_row[:], in_=rs_ps[:])
            rb_ps = psA.tile([128, NLOC], f32, tag="s")
            nc.tensor.matmul(rb_ps[:], ones_f32[:], rinv_row[:], start=True, stop=True)
            rb_sb = persist.tile([128, NLOC], f32, tag="rb_sb")
            nc.vector.tensor_copy(out=rb_sb[:], in_=rb_ps[:])

            y = [persist.tile([128, NLOC], f16, tag=f"y{dt}", name=f"y{dt}")
                 for dt in range(CG)]
            for dt in range(CG):
                nc.vector.tensor_tensor(out=y[dt][:], in0=y_ps[dt][:], in1=rb_sb[:],
                                        op=MUL)

            # ---- phase 3: out = x + upsample8(y) ----
            # loads on sync ring, adds on DVE, stores on scalar ring. x3/yup
            # share the phase-1 "x1" slots: the loads recycle those buffers as
            # pooling drains, which times the prefetch into the exchange gap.
            for cg in range(CG):
                yup = p1.tile([128, IL, W], f16, tag="x1", name=f"yup{cg}")
                nc.scalar.copy(
                    out=yup[:].rearrange("p i (j z) -> p i j z", z=DS),
                    in_=y[cg][:].rearrange("p (i j) -> p i j", i=IL)
                        [:, :, :, None].broadcast_to([128, IL, WP, DS]))
                for it in range(P1T):
                    x3 = p1.tile([128, TROWS, W], f16, tag="x1", name=f"x3_{cg}_{it}")
                    nc.sync.dma_start(
                        out=x3[:],
                        in_=x_ext.ap()[cg * 128:(cg + 1) * 128,
                                       it * TROWS:(it + 1) * TROWS, :])
                    for i in range(TROWS // DS):
                        xv = x3[:, i * DS:(i + 1) * DS, :]
                        yv = yup[:, it * 4 + i:it * 4 + i + 1, :] \
                            .broadcast_to([128, DS, W])
                        nc.vector.tensor_tensor(out=xv, in0=xv, in1=yv, op=ADD)
                    nc.scalar.dma_start(
                        out=out_ext.ap()[cg * 128:(cg + 1) * 128,
                                         it * TROWS:(it + 1) * TROWS, :],
                        in_=x3[:])

    nc.finalize()
    return nc


def _get_nc():
    if "nc" not in _CACHE:
        _CACHE["nc"] = _build()
    return _CACHE["nc"]


def kernel(x, Wq, bq, Wk, bk, Wv, bv):
    global LAST_EXEC_NS
    from concourse.bass_utils import run_bass_kernel_spmd

    # x round-trips HBM three times (pool read, residual read, output
    # write); fp16 halves that traffic and its ~5e-4 element error is far
    # inside the output envelope (the attention correction is ~2% of the
    # output norm, and the tolerance is 2e-2).
    x = np.asarray(x, dtype=np.float32).astype(np.float16)
    Wq = np.asarray(Wq, dtype=np.float32)
    bq = np.asarray(bq, dtype=np.float32).reshape(1, K)
    Wk = np.asarray(Wk, dtype=np.float32)
    bk = np.asarray(bk, dtype=np.float32).reshape(1, K)
    Wv = np.asarray(Wv, dtype=np.float32)
    bv = np.asarray(bv, dtype=np.float32).reshape(1, C)

    nc = _get_nc()
    in_maps = []
    for core in range(8):
        b, half = core // 2, core % 2
        in_maps.append({
            "x": np.ascontiguousarray(x[b, :, half * HL:(half + 1) * HL, :]),
            "wq": Wq, "bq": bq, "wk": Wk, "bk": bk, "wv": Wv, "bv": bv,
        })

    res = run_bass_kernel_spmd(nc, in_maps, core_ids=list(range(8)), trace=TRACE,
                               tmpdir=os.environ.get("BAM_TMPDIR") or None)
    LAST_EXEC_NS = res.exec_time_ns

    out = np.empty((B, C, H, W), dtype=np.float32)
    for core in range(8):
        b, half = core // 2, core % 2
        out[b, :, half * HL:(half + 1) * HL, :] = res.results[core]["out"]
    return out


# revision 50
# speedup vs baseline: 1.0381x; 1.0052x over previous
"""BAM-style attention block (avgpool8 -> 1024-token attention -> nearest-upsample + residual)
as a distributed Bass kernel on 8 TRN2 NeuronCores.

Sharding: core = b*2 + half  (b = batch 0..3, half = H-half 0..1).
x streams through HBM in fp16 (host casts; the output is dominated by the
residual x, so fp16's ~5e-4 element error is far inside the 2e-2 envelope).

Per core:
  phase 1: streams its x shard [512, 128, 256] in 2MB tiles (sync ring) and
           8x8-pools on DVE as a pairwise row-sum tree (TENSOR_TENSOR stays in
           the DVE 2-byte fast mode; TENSOR_REDUCE does not) + one small
           column-group reduce. Raw block SUMS (~N(0,64)) are kept in fp16;
           the 1/64 pool scale is folded into the q/k/v weight tiles, so the
           pooled data is consumed by the PE with no rescale pass at all.
  exchange: two pairwise AllGathers of the raw pooled sums (one per
           channel-group pair; each collective costs ~20us regardless of
           size, so the first is fired at P1 halftime and hides, only the
           second is exposed). The partner half is never materialized:
           k/v are linear in the pooled input, so remote tiles come from
           matmul-accumulating BOTH gathered halves (+2b bias) and
           subtracting the already-computed local tile during the PSUM
           drain - the gpsimd stream carries nothing but staging DMAs and
           collective triggers (a gpsimd compute op costs a Q7 library swap
           and lets the Tile scheduler head-of-line block the collectives).
  phase 2: 512x1024 attention, all-fp16 operands. Energies are computed
           TRANSPOSED (eT[n,m] = k^T q, one matmul per 128-token source tile)
           so exp() lands directly in the layout the y-contraction needs -
           no PE transposes. Softmax normalization is deferred to a row-sum
           rescale of y. The local token half is emitted (and scheduled)
           ahead of everything exchange-dependent.
  phase 3: re-streams x in 2MB tiles, adds the upsampled attention output on
           DVE (y is pre-upsampled along W on the idle ACT engine so the DVE
           add keeps a packed innermost axis), writes out (scalar ring).
           Phase-3 tiles share the phase-1 tile slots, which both bounds SBUF
           and naturally delays the prefetch until pooling drains - the
           loads then fill the exchange-latency gap instead of competing
           with phase-1 streaming.
"""

import os
import numpy as np

B, C, H, W = 4, 512, 256, 256
DS = 8
HL = H // 2            # 128 rows per core
IL = HL // DS          # 16 pooled rows per core
WP = W // DS           # 32 pooled cols
NLOC = IL * WP         # 512 local tokens
N = 2 * NLOC           # 1024 tokens
K = C // 8             # 64
CG = C // 128          # 4 channel groups
NT = N // 128          # 8 token tiles (0..3 local, 4..7 remote)
TROWS = 32             # x rows per streaming tile (= 4 pooled rows)
P1T = HL // TROWS      # 4 tiles per channel group in phase 1/3
QTOK = 4 * WP          # 128 tokens staged per tile (exchange quarter)

_CACHE = {}
TRACE = bool(int(os.environ.get("BAM_TRACE", "0")))
LAST_EXEC_NS = None


def _build():
    import concourse.bass as bass
    import concourse.tile as tile
    from concourse import bacc, mybir
    from concourse.masks import make_identity

    f32 = mybir.dt.float32
    f16 = mybir.dt.float16
    ADD = mybir.AluOpType.add
    SUB = mybir.AluOpType.subtract
    MUL = mybir.AluOpType.mult
    Copy = mybir.ActivationFunctionType.Copy
    Exp = mybir.ActivationFunctionType.Exp
    POOL_SCALE = 1.0 / (DS * DS)

    nc = bacc.Bacc("TRN2", target_bir_lowering=False, debug=False, num_devices=8)

    x_ext = nc.dram_tensor("x", [C, HL, W], f16, kind="ExternalInput")
    wq_ext = nc.dram_tensor("wq", [K, C], f32, kind="ExternalInput")
    bq_ext = nc.dram_tensor("bq", [1, K], f32, kind="ExternalInput")
    wk_ext = nc.dram_tensor("wk", [K, C], f32, kind="ExternalInput")
    bk_ext = nc.dram_tensor("bk", [1, K], f32, kind="ExternalInput")
    wv_ext = nc.dram_tensor("wv", [C, C], f32, kind="ExternalInput")
    bv_ext = nc.dram_tensor("bv", [1, C], f32, kind="ExternalInput")
    out_ext = nc.dram_tensor("out", [C, HL, W], f16, kind="ExternalOutput")

    with tile.TileContext(nc) as tc:
        with tc.tile_pool(name="persist", bufs=1) as persist, \
             tc.tile_pool(name="scratch", bufs=2) as scratch, \
             tc.tile_pool(name="p1", bufs=7) as p1, \
             tc.tile_pool(name="psA", bufs=4, space="PSUM") as psA, \
             tc.tile_pool(name="psY", bufs=1, space="PSUM") as psY, \
             tc.tile_pool(name="dram", bufs=1, space="DRAM") as dram:

            # ---- constants & weights (scalar-engine DMA ring; PE transposes).
            # All weight tiles carry the 1/64 pooling scale so every consumer
            # of pooled data reads the raw fp16 block sums directly.
            ident = persist.tile([128, 128], f16, tag="ident")
            make_identity(nc, ident[:])
            ones = persist.tile([1, N], f16, tag="ones")
            nc.vector.memset(ones[:], 1.0)
            ones_col = persist.tile([128, 1], f16, tag="ones_col")
            nc.vector.memset(ones_col[:], 1.0)
            ones_f32 = persist.tile([1, 128], f32, tag="ones_f32")
            nc.vector.memset(ones_f32[:], 1.0)

            def load_bias(ext, n):
                st = scratch.tile([1, n], f32, tag="bstage")
                nc.scalar.dma_start(out=st[:], in_=ext.ap())
                bb = persist.tile([1, n], f16, tag=f"b_{ext.name}", name=f"b_{ext.name}")
                nc.scalar.copy(out=bb[:], in_=st[:])
                return bb

            bq_b = load_bias(bq_ext, K)
            bk_b = load_bias(bk_ext, K)
            bv_b = load_bias(bv_ext, C)
            # doubled biases for the pair-sum algebra:
            # W*(h0+h1)*s + 2b - (W*xf*s + b) = W*xf_partner*s + b
            bk2_b = persist.tile([1, K], f16, tag="bk2")
            nc.vector.tensor_scalar_mul(bk2_b[:], bk_b[:], 2.0)
            bv2_b = persist.tile([1, C], f16, tag="bv2")
            nc.vector.tensor_scalar_mul(bv2_b[:], bv_b[:], 2.0)

            def load_qk_weight(ext):
                st = scratch.tile([K, C], f32, tag="wstage")
                nc.scalar.dma_start(out=st[:], in_=ext.ap())
                wb = persist.tile([K, C], f16, tag=f"wb_{ext.name}", name=f"wb_{ext.name}")
                nc.scalar.copy(out=wb[:], in_=st[:])
                wT = []
                for cg in range(CG):
                    ps = psA.tile([128, K], f16, tag="s")
                    nc.tensor.transpose(ps[:], wb[:, cg * 128:(cg + 1) * 128],
                                        ident[0:K, 0:K])
                    t = persist.tile([128, K], f16, tag=f"wT_{ext.name}{cg}",
                                     name=f"wT_{ext.name}{cg}")
                    nc.scalar.activation(out=t[:], in_=ps[:], func=Copy,
                                         scale=POOL_SCALE)
                    wT.append(t)
                return wT

            wqT = load_qk_weight(wq_ext)
            wkT = load_qk_weight(wk_ext)

            # wvT[cg][c_loc, d] = Wv[d, cg*128 + c_loc] / 64
            wvT = [persist.tile([128, C], f16, tag=f"wvT{cg}", name=f"wvT{cg}")
                   for cg in range(CG)]
            for dt in range(CG):
                st = scratch.tile([128, C], f32, tag="wstage")
                nc.scalar.dma_start(out=st[:], in_=wv_ext.ap()[dt * 128:(dt + 1) * 128, :])
                wvb = scratch.tile([128, C], f16, tag="wvstage")
                nc.scalar.copy(out=wvb[:], in_=st[:])
                for cg in range(CG):
                    ps = psA.tile([128, 128], f16, tag="s")
                    nc.tensor.transpose(ps[:], wvb[:, cg * 128:(cg + 1) * 128], ident[:])
                    nc.scalar.activation(out=wvT[cg][:, dt * 128:(dt + 1) * 128],
                                         in_=ps[:], func=Copy, scale=POOL_SCALE)

            # ---- phase 1: stream x + pool; pairwise exchange on the gpsimd ring ----
            # Tokens stay LOCAL-FIRST through phase 2: token tiles 0..3 are this
            # core's, 4..7 the partner's. Softmax and the final contraction are
            # permutation-invariant over n, so the global order is never
            # materialized.
            xf = [persist.tile([128, NLOC], f16, tag=f"xf{cg}", name=f"xf{cg}")
                  for cg in range(CG)]
            # hsg[cg][hf]: the two gathered halves of the pair (raw sums); one
            # of them IS this core's xf - never disambiguated (rank-agnostic).
            # hs01[cg] = h0 + h1, summed once on the (idle) DVE so the remote
            # k/v chains cost one matmul per channel group, not two.
            hsg = [[persist.tile([128, NLOC], f16, tag=f"hsg{cg}_{hf}",
                                 name=f"hsg{cg}_{hf}") for hf in range(2)]
                   for cg in range(CG)]
            hs01 = [persist.tile([128, NLOC], f16, tag=f"hs01_{cg}",
                                 name=f"hs01_{cg}") for cg in range(CG)]
            xf_loc_d = dram.tile([2, 128, 2 * NLOC], f16, tag="xf_loc")
            xf_all_d = dram.tile([2, 2, 128, 2 * NLOC], f16, tag="xf_all")

            q_ps = psA.tile([K, NLOC], f32, tag="s")
            kl_ps = psA.tile([K, NLOC], f32, tag="s")
            kr_ps = psA.tile([K, NLOC], f32, tag="s")

            lp = nc.allow_low_precision("8x8 block sums are ~N(0,64); fp16 "
                                        "keeps DVE in its 2-byte fast mode")
            lp.__enter__()
            HT = TROWS // 2  # 16, 8, 4: pairwise row-sum tree widths
            for cg in range(CG):
                for it in range(P1T):
                    x1 = p1.tile([128, TROWS, W], f16, tag="x1")
                    nc.sync.dma_start(
                        out=x1[:],
                        in_=x_ext.ap()[cg * 128:(cg + 1) * 128,
                                       it * TROWS:(it + 1) * TROWS, :])
                    s1 = p1.tile([128, HT, W], f16, tag="s1", bufs=1,
                                 name=f"s1_{cg}_{it}")
                    v0 = x1[:].rearrange("p (a b) w -> p a b w", b=2)
                    nc.vector.tensor_tensor(out=s1[:], in0=v0[:, :, 0, :],
                                            in1=v0[:, :, 1, :], op=ADD)
                    s2 = p1.tile([128, HT // 2, W], f16, tag="s2", bufs=1,
                                 name=f"s2_{cg}_{it}")
                    v1 = s1[:].rearrange("p (a b) w -> p a b w", b=2)
                    nc.vector.tensor_tensor(out=s2[:], in0=v1[:, :, 0, :],
                                            in1=v1[:, :, 1, :], op=ADD)
                    s3 = p1.tile([128, HT // 4, W], f16, tag="s3", bufs=1,
                                 name=f"s3_{cg}_{it}")
                    v2 = s2[:].rearrange("p (a b) w -> p a b w", b=2)
                    nc.vector.tensor_tensor(out=s3[:], in0=v2[:, :, 0, :],
                                            in1=v2[:, :, 1, :], op=ADD)
                    nc.vector.tensor_reduce(
                        out=xf[cg][:, it * QTOK:(it + 1) * QTOK]
                            .rearrange("p (r j) -> p r j", j=WP),
                        in_=s3[:].rearrange("p r (j z) -> p r j z", z=DS),
                        axis=mybir.AxisListType.X, op=ADD)
                # one staging DMA per channel group: fewer semaphore checks
                # ahead of the collective trigger than per-quarter staging
                nc.gpsimd.dma_start(
                    out=xf_loc_d[cg // 2][:, (cg % 2) * NLOC:
                                          (cg % 2 + 1) * NLOC],
                    in_=xf[cg][:])

                # local q/k partials (rhs = raw sums; scale lives in wqT/wkT)
                nc.tensor.matmul(q_ps[:], wqT[cg][:], xf[cg][:],
                                 start=(cg == 0), stop=False)
                nc.tensor.matmul(kl_ps[:], wkT[cg][:], xf[cg][:],
                                 start=(cg == 0), stop=False)
                if cg % 2 == 1:
                    nc.gpsimd.collective_compute(
                        "AllGather",
                        mybir.AluOpType.bypass,
                        ins=[xf_loc_d[cg // 2].opt()],
                        outs=[xf_all_d[cg // 2].opt()],
                        replica_groups=[[0, 1], [2, 3], [4, 5], [6, 7]],
                    )
            lp.__exit__(None, None, None)

            # ================= LOCAL attention half =================
            # Emitted (hence per-engine scheduled) BEFORE anything that waits
            # on a collective: it runs while the second AllGather is in
            # flight.
            nc.tensor.matmul(q_ps[:], bq_b[:], ones[:, :NLOC], start=False, stop=True)
            q_sb = persist.tile([K, NLOC], f16, tag="q_sb")
            nc.vector.tensor_copy(out=q_sb[:], in_=q_ps[:])
            nc.tensor.matmul(kl_ps[:], bk_b[:], ones[:, :NLOC], start=False, stop=True)
            k_loc = persist.tile([K, NLOC], f16, tag="k_loc")
            nc.vector.tensor_copy(out=k_loc[:], in_=kl_ps[:])

            vT = [persist.tile([128, C], f16, tag=f"vT{nt}", name=f"vT{nt}")
                  for nt in range(NT)]
            attnT = [persist.tile([128, NLOC], f16, tag=f"attnT{nt}",
                                  name=f"attnT{nt}")
                     for nt in range(NT)]
            y_ps = [psY.tile([128, NLOC], f32, tag=f"y{dt}", name=f"yps{dt}")
                    for dt in range(CG)]
            rs_ps = psA.tile([1, NLOC], f32, tag="s", name="rs_ps")

            def vt_tile(nt):
                # vT[nt][n, d] = v[d, n] for token tile nt (128 tokens).
                # Remote tiles accumulate BOTH gathered halves (+2bv) and
                # subtract the local tile while draining the PSUM.
                j = nt % 4
                v_ps = psA.tile([128, C], f32, tag="s", name=f"v_ps{nt}")
                if nt < 4:
                    for cg in range(CG):
                        nc.tensor.matmul(v_ps[:], xf[cg][:, j * 128:(j + 1) * 128],
                                         wvT[cg][:], start=(cg == 0), stop=False)
                    nc.tensor.matmul(v_ps[:], ones[:, :128], bv_b[:],
                                     start=False, stop=True)
                    nc.vector.tensor_copy(out=vT[nt][:], in_=v_ps[:])
                else:
                    for cg in range(CG):
                        nc.tensor.matmul(
                            v_ps[:], hs01[cg][:, j * 128:(j + 1) * 128],
                            wvT[cg][:], start=(cg == 0), stop=False)
                    nc.tensor.matmul(v_ps[:], ones[:, :128], bv2_b[:],
                                     start=False, stop=True)
                    nc.vector.tensor_tensor(out=vT[nt][:], in0=v_ps[:],
                                            in1=vT[nt - 4][:], op=SUB)

            # attnT holds UNNORMALIZED exp(eT/sqrt(K)); normalization is a
            # final row-sum rescale of y. Energies are tiny (|e/sqrt(K)| <
            # ~0.05) so exp without max-subtraction is safe. eT[n, m] =
            # sum_K k[K, n] q[K, m] leaves the PE already transposed.
            def attn_tile(nt):
                ksb = k_loc if nt < 4 else k_rem
                j = nt % 4
                e_ps = psA.tile([128, NLOC], f32, tag="s", name=f"e_ps{nt}")
                nc.tensor.matmul(e_ps[:], ksb[:, j * 128:(j + 1) * 128], q_sb[:],
                                 start=True, stop=True)
                nc.scalar.activation(out=attnT[nt][:], in_=e_ps[:], func=Exp,
                                     scale=K ** -0.5)

            def y_accum(nt):
                # y_raw[d, m] += sum_n v[d, n] a[m, n]; rowsum[m] += sum_n a[m, n]
                for dt in range(CG):
                    nc.tensor.matmul(y_ps[dt][:], vT[nt][:, dt * 128:(dt + 1) * 128],
                                     attnT[nt][:], start=(nt == 0),
                                     stop=(nt == NT - 1))
                nc.tensor.matmul(rs_ps[:], ones_col[:], attnT[nt][:],
                                 start=(nt == 0), stop=(nt == NT - 1))

            for nt in range(4):
                vt_tile(nt)
                attn_tile(nt)
                y_accum(nt)

            # ================= REMOTE attention half =================
            def load_pair(pr):
                for cg in (2 * pr, 2 * pr + 1):
                    sl = (cg % 2) * NLOC
                    for hf in range(2):
                        nc.scalar.dma_start(
                            out=hsg[cg][hf][:],
                            in_=xf_all_d[pr, hf][:, sl:sl + NLOC])
                    nc.vector.tensor_tensor(out=hs01[cg][:], in0=hsg[cg][0][:],
                                            in1=hsg[cg][1][:], op=ADD)
                    nc.tensor.matmul(kr_ps[:], wkT[cg][:], hs01[cg][:],
                                     start=(cg == 0), stop=False)

            load_pair(0)
            load_pair(1)
            nc.tensor.matmul(kr_ps[:], bk2_b[:], ones[:, :NLOC], start=False,
                             stop=True)
            k_rem = persist.tile([K, NLOC], f16, tag="k_rem")
            nc.vector.tensor_tensor(out=k_rem[:], in0=kr_ps[:], in1=k_loc[:],
                                    op=SUB)

            for nt in range(4, NT):
                vt_tile(nt)
                attn_tile(nt)
                y_accum(nt)

            # softmax denominators -> broadcast rescale of y. The [1,512] row
            # is serial one-lane work; the ~18-bit fast approximation (5x
            # faster than full reciprocal) is far more accurate than needed.
            rinv_row = persist.tile([1, NLOC], f32, tag="rinv_row")
            nc.vector.reciprocal_approx_fast(out=rinv_row[:], in_=rs_ps[:])
            rb_ps = psA.tile([128, NLOC], f32, tag="s")
            nc.tensor.matmul(rb_ps[:], ones_f32[:], rinv_row[:], start=True, stop=True)
            rb_sb = persist.tile([128, NLOC], f32, tag="rb_sb")
            nc.vector.tensor_copy(out=rb_sb[:], in_=rb_ps[:])

            y = [persist.tile([128, NLOC], f16, tag=f"y{dt}", name=f"y{dt}")
                 for dt in range(CG)]
            for dt in range(CG):
                nc.vector.tensor_tensor(out=y[dt][:], in0=y_ps[dt][:], in1=rb_sb[:],
                                        op=MUL)

            # ---- phase 3: out = x + upsample8(y) ----
            # loads on sync ring, adds on DVE, stores on scalar ring. x3/yup
            # share the phase-1 "x1" slots: the loads recycle those buffers as
            # pooling drains, which times the prefetch into the exchange gap.
            for cg in range(CG):
                yup = p1.tile([128, IL, W], f16, tag="x1", name=f"yup{cg}")
                nc.scalar.copy(
                    out=yup[:].rearrange("p i (j z) -> p i j z", z=DS),
                    in_=y[cg][:].rearrange("p (i j) -> p i j", i=IL)
                        [:, :, :, None].broadcast_to([128, IL, WP, DS]))
                for it in range(P1T):
                    x3 = p1.tile([128, TROWS, W], f16, tag="x1", name=f"x3_{cg}_{it}")
                    nc.sync.dma_start(
                        out=x3[:],
                        in_=x_ext.ap()[cg * 128:(cg + 1) * 128,
                                       it * TROWS:(it + 1) * TROWS, :])
                    for i in range(TROWS // DS):
                        xv = x3[:, i * DS:(i + 1) * DS, :]
                        yv = yup[:, it * 4 + i:it * 4 + i + 1, :] \
                            .broadcast_to([128, DS, W])
                        nc.vector.tensor_tensor(out=xv, in0=xv, in1=yv, op=ADD)
                    nc.scalar.dma_start(
                        out=out_ext.ap()[cg * 128:(cg + 1) * 128,
                                         it * TROWS:(it + 1) * TROWS, :],
                        in_=x3[:])

    nc.finalize()
    return nc


def _get_nc():
    if "nc" not in _CACHE:
        _CACHE["nc"] = _build()
    return _CACHE["nc"]


def kernel(x, Wq, bq, Wk, bk, Wv, bv):
    global LAST_EXEC_NS
    from concourse.bass_utils import run_bass_kernel_spmd

    # x round-trips HBM three times (pool read, residual read, output
    # write); fp16 halves that traffic and its ~5e-4 element error is far
    # inside the output envelope (the attention correction is ~2% of the
    # output norm, and the tolerance is 2e-2).
    x = np.asarray(x, dtype=np.float32).astype(np.float16)
    Wq = np.asarray(Wq, dtype=np.float32)
    bq = np.asarray(bq, dtype=np.float32).reshape(1, K)
    Wk = np.asarray(Wk, dtype=np.float32)
    bk = np.asarray(bk, dtype=np.float32).reshape(1, K)
    Wv = np.asarray(Wv, dtype=np.float32)
    bv = np.asarray(bv, dtype=np.float32).reshape(1, C)

    nc = _get_nc()
    in_maps = []
    for core in range(8):
        b, half = core // 2, core % 2
        in_maps.append({
            "x": np.ascontiguousarray(x[b, :, half * HL:(half + 1) * HL, :]),
            "wq": Wq, "bq": bq, "wk": Wk, "bk": bk, "wv": Wv, "bv": bv,
        })

    res = run_bass_kernel_spmd(nc, in_maps, core_ids=list(range(8)), trace=TRACE,
                               tmpdir=os.environ.get("BAM_TMPDIR") or None)
    LAST_EXEC_NS = res.exec_time_ns

    out = np.empty((B, C, H, W), dtype=np.float32)
    for core in range(8):
        b, half = core // 2, core % 2
        out[b, :, half * HL:(half + 1) * HL, :] = res.results[core]["out"]
    return out
